# revision 21
# baseline (speedup 1.0000x reference)
"""Weighted-KNN (retrieval_knn) Trainium2 kernel — candidate-pruned, gather-free.

Math (per query c, over N anchors):
    sq[n]   = ||c - p_n||^2 / (w_n^2 + eps)
    top-8 smallest sq -> softmax(-sq_k / TEMP) -> weighted sum of features.

Strategy:
  HOST (numpy, in kernel()):
    * kd-split the 65536 queries into 512 spatial tiles of 128.
    * Per tile, an exact interval bound (f64) selects the candidate anchors
      that can possibly be in ANY tile query's top-8:
          T8 = 8th-smallest over anchors of max_{x in bbox} eff(x,n),
          keep n with min_{x in bbox} eff(x,n) <= T8.
      Mean ~320 candidates instead of 16384 (~39x less score work).
    * Tiles are LPT-balanced across the 8 cores (64 slots each), sorted by
      candidate count, and padded to one shared static schedule so a single
      NEFF serves all cores. Host ships per-core tables:
          qrow  [64, 384]   tile query coords (centered), rows c0|c1|c2
          gtabT [SUML, 4]   per-candidate [g0, -p'0, -p'1, -p'2],
                            g0 = -INV_TEMP/(w^2+eps)
          feat  [SUML, 72]  [features(64), 1.0, pad(7)]
  DEVICE (per tile, all engines pipelined, no dma_gather anywhere):
    * crep = partition_broadcast of the tile's query rows (Pool).
    * Scores via EXACT direct differences (same precision class as the
      reference): sq_d = Square(crep_d + (-p'_d)) on ScalarE with
      per-partition bias; y^T[j,q] = (sq0+sq1+sq2) * g0_j  (DVE).
    * PE-transpose y^T -> y[q,j] in PSUM; top-8 per query via DVE max8
      (per-512 pre-max8 + combine; exact fp32, pigeonhole-safe).
    * Mask  = (y^T >= s_bcast) (Pool), W = exp(y^T + C) * Mask
      (ScalarE exp + Pool mult; C is a global shift, cancels in softmax).
    * Feature blend as accumulating matmuls: out[q,:65] = sum_j W^T f_j,
      column 64 of feat is 1.0 so out[:,64] = Z (self-consistent softmax).
    * out = psB[:, :64] * (1/Z); DMA to DRAM; host un-permutes rows.
"""

import sys

if "/opt/trn_rl_repo" not in sys.path:
    sys.path.insert(0, "/opt/trn_rl_repo")

import numpy as np

import concourse.bacc as bacc
import concourse.bass as bass
import concourse.mybir as mybir
from concourse.bass import ts
from concourse.bass_utils import run_bass_kernel_spmd
from concourse.tile import TileContext

B, N, D, F = 65536, 16384, 3, 64
K = 8
BANDWIDTH = 0.05
TEMP = 2.0 * BANDWIDTH * BANDWIDTH  # 0.005
INV_TEMP = 1.0 / TEMP  # 200.0
EPS = 1e-8
NCORES = 8
QPC = B // NCORES  # 8192 queries per core
P = 128
NSLOT = QPC // P  # 64 tiles per core
FE = F + 8  # feat row: 64 features, ones col, 7 pad
LOOP = 1  # in-NEFF repetitions of the whole tile loop (benchmarking)

FP = mybir.dt.float32
AF = mybir.ActivationFunctionType


# ---------------------------------------------------------------- host prep
def _kd_leaves(coords):
    def split(idx, depth):
        if len(idx) == P:
            return [idx]
        ax = depth % 3
        k = len(idx) // 2
        part = np.argpartition(coords[idx, ax], k)
        return split(idx[part[:k]], depth + 1) + split(idx[part[k:]], depth + 1)

    return split(np.arange(coords.shape[0]), 0)


def prep(coords, positions, weights, features):
    """Host-side index construction. Returns (in_maps_arrays, meta)."""
    coords = np.ascontiguousarray(coords, dtype=np.float32)
    p64 = np.ascontiguousarray(positions, dtype=np.float64)
    w64 = np.ascontiguousarray(weights, dtype=np.float64)
    features = np.ascontiguousarray(features, dtype=np.float32)
    inv64 = 1.0 / (w64 * w64 + EPS)

    leaves = _kd_leaves(coords)
    ntiles = len(leaves)
    cands, centers, counts, t8s = [], [], [], []
    for lf in leaves:
        c = coords[lf].astype(np.float64)
        lo, hi = c.min(0), c.max(0)
        dmin2 = (np.clip(np.maximum(lo - p64, p64 - hi), 0, None) ** 2).sum(1)
        dmax2 = (np.maximum((p64 - lo) ** 2, (p64 - hi) ** 2)).sum(1)
        emin, emax = dmin2 * inv64, dmax2 * inv64
        t8 = np.partition(emax, K - 1)[K - 1] * (1 + 1e-4) + 1e-9
        cl = np.where(emin <= t8)[0]
        assert len(cl) >= K
        cands.append(cl)
        centers.append((lo + hi) / 2)
        counts.append(len(cl))
        t8s.append(t8)
    counts = np.array(counts)
    t8s = np.array(t8s)
    assert INV_TEMP * t8s.max() <= 160.0, t8s.max()
    cshift = float(np.clip(INV_TEMP * t8s.max() - 40.0, 0.0, 80.0))

    # LPT-balance tiles onto cores (64 slots each), sort desc by count
    order = np.argsort(-counts, kind="stable")
    loads = np.zeros(NCORES)
    slots = [[] for _ in range(NCORES)]
    for t in order:
        free = [c for c in range(NCORES) if len(slots[c]) < NSLOT]
        c = min(free, key=lambda c: loads[c])
        slots[c].append(t)
        loads[c] += counts[t]
    for c in range(NCORES):
        slots[c].sort(key=lambda t: -counts[t])
    ls = np.array(
        [[counts[slots[c][j]] for j in range(NSLOT)] for c in range(NCORES)]
    )
    sched = np.maximum(P, ((ls.max(0) + P - 1) // P) * P).astype(np.int64)
    offs = np.concatenate([[0], np.cumsum(sched)])
    suml = int(offs[-1])

    per_core = []
    outperm = []
    for c in range(NCORES):
        qrow = np.zeros((NSLOT, 3 * P), np.float32)
        gtabT = np.zeros((suml, 4), np.float32)
        gtabT[:, 0] = -1.0
        gtabT[:, 1:4] = -100.0  # pad: far away, y ~ -3e4
        feat = np.zeros((suml, FE), np.float32)
        for j in range(NSLOT):
            t = slots[c][j]
            lf, cl, ctr = leaves[t], cands[t], centers[t]
            qc = (coords[lf].astype(np.float64) - ctr).astype(np.float32)
            qrow[j, :] = qc.T.reshape(-1)
            o, n = offs[j], len(cl)
            L = int(sched[j])
            nch = L // P
            # device reads tile slices as [128, nch*row] with partition p
            # holding DRAM rows [o + p*nch, o + (p+1)*nch) contiguously; host
            # permutes so candidate j = c*128 + p lands at row o + p*nch + c.
            gt = np.zeros((L, 4), np.float32)
            gt[:, 0] = -1.0
            gt[:, 1:4] = -100.0
            gt[:n, 0] = (-INV_TEMP * inv64[cl]).astype(np.float32)
            gt[:n, 1:4] = -(p64[cl] - ctr).astype(np.float32)
            fe = np.zeros((L, FE), np.float32)
            fe[:n, 0:F] = features[cl]
            fe[:n, F] = 1.0
            shuf = (np.arange(L).reshape(nch, P).T).reshape(-1)  # row p*nch+c <- cand c*128+p
            gtabT[o : o + L] = gt[shuf]
            feat[o : o + L] = fe[shuf]
            outperm.append(lf)
        per_core.append({"qrow": qrow, "gtabT": gtabT, "feat": feat})
    outperm = np.concatenate(outperm)
    inv_perm = np.empty(B, np.int64)
    inv_perm[outperm] = np.arange(B)
    meta = {
        "sched": tuple(int(x) for x in sched),
        "offs": offs,
        "suml": suml,
        "cshift": cshift,
        "inv_perm": inv_perm,
    }
    return per_core, meta


# ------------------------------------------------------------- device build
def _build_nc(sched, suml, cshift, loop=1):
    nc = bacc.Bacc("TRN2")
    qrow_in = nc.declare_dram_parameter("qrow", [NSLOT, 3 * P], FP, isOutput=False)
    gtabT_in = nc.declare_dram_parameter("gtabT", [suml, 4], FP, isOutput=False)
    feat_in = nc.declare_dram_parameter("feat", [suml, FE], FP, isOutput=False)
    ident_in = nc.declare_dram_parameter("ident", [P, P], FP, isOutput=False)
    out = nc.declare_dram_parameter("out", [QPC, F], FP, isOutput=True)

    offs = np.concatenate([[0], np.cumsum(sched)]).astype(np.int64)

    with TileContext(nc) as tc:
        with tc.tile_pool(name="const", bufs=1) as cpool:
            ident = cpool.tile([P, P], FP)
            nc.sync.dma_start(ident[:], ident_in[:])
            cbias = cpool.tile([P, 1], FP)
            nc.vector.memset(cbias[:], cshift)

            with (
                tc.tile_pool(name="io", bufs=3) as io,
                tc.tile_pool(name="work", bufs=2) as wk,
                tc.tile_pool(name="chk", bufs=4) as ck,
                tc.tile_pool(name="ps_y", bufs=3, space="PSUM") as psy,
                tc.tile_pool(name="ps_b", bufs=2, space="PSUM") as psb,
                tc.tile_pool(name="ps_s", bufs=2, space="PSUM") as pss,
            ):
                for it in range(NSLOT * loop):
                    t = it % NSLOT
                    L = int(sched[t])
                    o = int(offs[t])
                    nch = L // P  # 128-wide sub-chunks
                    ng = (L + 511) // 512  # 512-wide groups

                    # ---- loads ----
                    csrc = io.tile([1, 3 * P], FP, tag="csrc", name=f"cs{it}")
                    nc.sync.dma_start(csrc[:], qrow_in[t : t + 1, :])
                    crep = wk.tile([P, 3 * P], FP, tag="crep", name=f"cr{it}")
                    nc.gpsimd.partition_broadcast(crep[:], csrc[:])
                    pcol = io.tile([P, nch, 4], FP, tag="pcol", name=f"pc{it}")
                    nc.sync.dma_start(
                        pcol[:],
                        gtabT_in[o : o + L, :].rearrange("(p c) f -> p c f", p=P),
                    )
                    ft = io.tile([P, nch, FE], FP, tag="ft", name=f"ft{it}")
                    nc.sync.dma_start(
                        ft[:],
                        feat_in[o : o + L, :].rearrange("(p c) f -> p c f", p=P),
                    )

                    yT = wk.tile([P, nch, P], FP, tag="yT", name=f"yT{it}")
                    e8all = wk.tile([P, 8 * ng], FP, tag="e8all", name=f"e8a{it}")

                    # ---- phase A: scores (exact direct differences) ----
                    for g in range(ng):
                        gw = min(4, nch - 4 * g)
                        psY = psy.tile([P, 512], FP, tag="psY", name=f"psY{it}_{g}")
                        for kk in range(gw):
                            ci = 4 * g + kk
                            sqa = ck.tile([P, P], FP, tag="sqa", name=f"sqa{it}_{ci}")
                            sqb = ck.tile([P, P], FP, tag="sqb", name=f"sqb{it}_{ci}")
                            nc.scalar.activation(
                                sqa[:], crep[:, 0:P], AF.Square,
                                bias=pcol[:, ci, 1:2],
                            )
                            nc.scalar.activation(
                                sqb[:], crep[:, P : 2 * P], AF.Square,
                                bias=pcol[:, ci, 2:3],
                            )
                            nc.vector.tensor_add(sqa[:], sqa[:], sqb[:])
                            nc.scalar.activation(
                                sqb[:], crep[:, 2 * P : 3 * P], AF.Square,
                                bias=pcol[:, ci, 3:4],
                            )
                            nc.vector.tensor_add(sqa[:], sqa[:], sqb[:])
                            nc.vector.tensor_scalar_mul(
                                yT[:, ci, :], sqa[:], pcol[:, ci, 0:1]
                            )
                            nc.tensor.transpose(
                                psY[:, kk * P : (kk + 1) * P], yT[:, ci, :], ident[:]
                            )
                        nc.vector.max(e8all[:, 8 * g : 8 * g + 8], psY[:, 0 : gw * P])

                    # ---- combine top-8; broadcast threshold ----
                    e8 = ck.tile([P, 8], FP, tag="e8", name=f"e8{it}")
                    nc.vector.max(e8[:], e8all[:])
                    psS = pss.tile([1, P], FP, tag="psS", name=f"psS{it}")
                    nc.tensor.transpose(psS[:], e8[:, 7:8], ident[:])
                    srow = ck.tile([1, P], FP, tag="srow", name=f"sr{it}")
                    nc.scalar.copy(srow[:], psS[:])
                    srep = wk.tile([P, P], FP, tag="srep", name=f"srp{it}")
                    nc.gpsimd.partition_broadcast(srep[:], srow[:])

                    # ---- phase B: masked exp weights + feature blend ----
                    psB = psb.tile([P, F + 1], FP, tag="psB", name=f"psB{it}")
                    for g in range(ng):
                        gw = min(4, nch - 4 * g)
                        et = ck.tile([P, gw, P], FP, tag="et", name=f"e{it}_{g}")
                        nc.scalar.activation(
                            et[:], yT[:, 4 * g : 4 * g + gw, :], AF.Exp,
                            bias=cbias[:],
                        )
                        for kk in range(gw):
                            ci = 4 * g + kk
                            msk = ck.tile([P, P], FP, tag="msk", name=f"m{it}_{ci}")
                            nc.vector.tensor_tensor(
                                out=msk[:], in0=yT[:, ci, :], in1=srep[:],
                                op=mybir.AluOpType.is_ge,
                            )
                            nc.gpsimd.tensor_mul(et[:, kk, :], et[:, kk, :], msk[:])
                            nc.tensor.matmul(
                                psB[:],
                                et[:, kk, :],
                                ft[:, ci, 0 : F + 1],
                                start=(ci == 0),
                                stop=(ci == nch - 1),
                            )

                    # ---- normalize + store ----
                    ob = ck.tile([P, F + 1], FP, tag="ob", name=f"ob{it}")
                    nc.scalar.copy(ob[:], psB[:])
                    rs = ck.tile([P, 1], FP, tag="rs", name=f"rs{it}")
                    nc.vector.reciprocal(rs[:], ob[:, F : F + 1])
                    ot = ck.tile([P, F], FP, tag="ot", name=f"ot{it}")
                    nc.vector.tensor_scalar_mul(ot[:], ob[:, 0:F], rs[:])
                    nc.sync.dma_start(out[ts(t, P), :], ot[:])

    nc.compile()
    return nc


# ------------------------------------------------------------------ runtime
_CACHE = {}


def _get_nc(sched, suml, cshift, loop):
    key = (sched, suml, round(cshift, 6), loop)
    if key not in _CACHE:
        _CACHE[key] = _build_nc(sched, suml, cshift, loop=loop)
    return _CACHE[key]


def make_in_maps(per_core):
    ident = np.eye(P, dtype=np.float32)
    return [
        {
            "qrow": pc["qrow"],
            "gtabT": pc["gtabT"],
            "feat": pc["feat"],
            "ident": ident,
        }
        for pc in per_core
    ]


LAST_RESULT = None


def kernel(coords, positions, weights, features):
    global LAST_RESULT
    import os

    per_core, meta = prep(coords, positions, weights, features)
    nc = _get_nc(meta["sched"], meta["suml"], meta["cshift"], LOOP)
    in_maps = make_in_maps(per_core)
    trace = bool(int(os.environ.get("KNN_TRACE", "0")))
    res = run_bass_kernel_spmd(nc, in_maps, core_ids=list(range(NCORES)), trace=trace)
    LAST_RESULT = res
    full = np.concatenate([res.results[i]["out"] for i in range(NCORES)], axis=0)
    return full[meta["inv_perm"]]


# revision 22
# speedup vs baseline: 1.1871x; 1.1871x over previous
"""Weighted-KNN (retrieval_knn) Trainium2 kernel — candidate-pruned, gather-free.

Math (per query c, over N anchors):
    sq[n]   = ||c - p_n||^2 / (w_n^2 + eps)
    top-8 smallest sq -> softmax(-sq_k / TEMP) -> weighted sum of features.

Strategy:
  HOST (numpy, in kernel()):
    * kd-split the 65536 queries into 512 spatial tiles of 128.
    * Per tile, an exact interval bound (f64) selects the candidate anchors
      that can possibly be in ANY tile query's top-8:
          T8 = 8th-smallest over anchors of max_{x in bbox} eff(x,n),
          keep n with min_{x in bbox} eff(x,n) <= T8.
      Mean ~320 candidates instead of 16384 (~39x less score work).
    * Tiles are LPT-balanced across the 8 cores (64 slots each), sorted by
      candidate count, and padded to one shared static schedule so a single
      NEFF serves all cores. Host ships per-core tables:
          qrow  [64, 384]   tile query coords (centered), rows c0|c1|c2
          gtabT [SUML, 4]   per-candidate [g0, -p'0, -p'1, -p'2],
                            g0 = -INV_TEMP/(w^2+eps)
          feat  [SUML, 72]  [features(64), 1.0, pad(7)]
  DEVICE (per tile, all engines pipelined, no dma_gather anywhere):
    * crep = partition_broadcast of the tile's query rows (Pool).
    * Scores via EXACT direct differences (same precision class as the
      reference): sq_d = Square(crep_d + (-p'_d)) on ScalarE with
      per-partition bias; y^T[j,q] = (sq0+sq1+sq2) * g0_j  (DVE).
    * PE-transpose y^T -> y[q,j] in PSUM; top-8 per query via DVE max8
      (per-512 pre-max8 + combine; exact fp32, pigeonhole-safe).
    * Mask  = (y^T >= s_bcast) (Pool), W = exp(y^T + C) * Mask
      (ScalarE exp + Pool mult; C is a global shift, cancels in softmax).
    * Feature blend as accumulating matmuls: out[q,:65] = sum_j W^T f_j,
      column 64 of feat is 1.0 so out[:,64] = Z (self-consistent softmax).
    * out = psB[:, :64] * (1/Z); DMA to DRAM; host un-permutes rows.
"""

import sys

if "/opt/trn_rl_repo" not in sys.path:
    sys.path.insert(0, "/opt/trn_rl_repo")

import numpy as np

import concourse.bacc as bacc
import concourse.bass as bass
import concourse.mybir as mybir
from concourse.bass import ts
from concourse.bass_utils import run_bass_kernel_spmd
from concourse.tile import TileContext

B, N, D, F = 65536, 16384, 3, 64
K = 8
BANDWIDTH = 0.05
TEMP = 2.0 * BANDWIDTH * BANDWIDTH  # 0.005
INV_TEMP = 1.0 / TEMP  # 200.0
EPS = 1e-8
NCORES = 8
QPC = B // NCORES  # 8192 queries per core
P = 128
NSLOT = QPC // P  # 64 tiles per core
FE = F + 8  # feat row: 64 features, ones col, 7 pad
LOOP = 1  # in-NEFF repetitions of the whole tile loop (benchmarking)

FP = mybir.dt.float32
AF = mybir.ActivationFunctionType


# ---------------------------------------------------------------- host prep
def _kd_leaves(coords):
    def split(idx, depth):
        if len(idx) == P:
            return [idx]
        ax = depth % 3
        k = len(idx) // 2
        part = np.argpartition(coords[idx, ax], k)
        return split(idx[part[:k]], depth + 1) + split(idx[part[k:]], depth + 1)

    return split(np.arange(coords.shape[0]), 0)


def prep(coords, positions, weights, features):
    """Host-side index construction. Returns (in_maps_arrays, meta)."""
    coords = np.ascontiguousarray(coords, dtype=np.float32)
    p64 = np.ascontiguousarray(positions, dtype=np.float64)
    w64 = np.ascontiguousarray(weights, dtype=np.float64)
    features = np.ascontiguousarray(features, dtype=np.float32)
    inv64 = 1.0 / (w64 * w64 + EPS)

    leaves = _kd_leaves(coords)
    ntiles = len(leaves)
    cands, centers, counts, t8s = [], [], [], []
    for lf in leaves:
        c = coords[lf].astype(np.float64)
        lo, hi = c.min(0), c.max(0)
        dmin2 = (np.clip(np.maximum(lo - p64, p64 - hi), 0, None) ** 2).sum(1)
        dmax2 = (np.maximum((p64 - lo) ** 2, (p64 - hi) ** 2)).sum(1)
        emin, emax = dmin2 * inv64, dmax2 * inv64
        t8 = np.partition(emax, K - 1)[K - 1] * (1 + 1e-4) + 1e-9
        cl = np.where(emin <= t8)[0]
        assert len(cl) >= K
        cands.append(cl)
        centers.append((lo + hi) / 2)
        counts.append(len(cl))
        t8s.append(t8)
    counts = np.array(counts)
    t8s = np.array(t8s)
    assert INV_TEMP * t8s.max() <= 160.0, t8s.max()
    cshift = float(np.clip(INV_TEMP * t8s.max() - 40.0, 0.0, 80.0))

    # LPT-balance tiles onto cores (64 slots each), sort desc by count
    order = np.argsort(-counts, kind="stable")
    loads = np.zeros(NCORES)
    slots = [[] for _ in range(NCORES)]
    for t in order:
        free = [c for c in range(NCORES) if len(slots[c]) < NSLOT]
        c = min(free, key=lambda c: loads[c])
        slots[c].append(t)
        loads[c] += counts[t]
    for c in range(NCORES):
        slots[c].sort(key=lambda t: -counts[t])
    ls = np.array(
        [[counts[slots[c][j]] for j in range(NSLOT)] for c in range(NCORES)]
    )
    sched = np.maximum(P, ((ls.max(0) + P - 1) // P) * P).astype(np.int64)
    offs = np.concatenate([[0], np.cumsum(sched)])
    suml = int(offs[-1])

    per_core = []
    outperm = []
    for c in range(NCORES):
        qrow = np.zeros((NSLOT, 3 * P), np.float32)
        gtabT = np.zeros((suml, 4), np.float32)
        gtabT[:, 0] = -1.0
        gtabT[:, 1:4] = -100.0  # pad: far away, y ~ -3e4
        feat = np.zeros((suml, FE), np.float32)
        for j in range(NSLOT):
            t = slots[c][j]
            lf, cl, ctr = leaves[t], cands[t], centers[t]
            qc = (coords[lf].astype(np.float64) - ctr).astype(np.float32)
            qrow[j, :] = qc.T.reshape(-1)
            o, n = offs[j], len(cl)
            L = int(sched[j])
            nch = L // P
            # device reads tile slices as [128, nch*row] with partition p
            # holding DRAM rows [o + p*nch, o + (p+1)*nch) contiguously; host
            # permutes so candidate j = c*128 + p lands at row o + p*nch + c.
            gt = np.zeros((L, 4), np.float32)
            gt[:, 0] = -1.0
            gt[:, 1:4] = -100.0
            gt[:n, 0] = (-INV_TEMP * inv64[cl]).astype(np.float32)
            gt[:n, 1:4] = -(p64[cl] - ctr).astype(np.float32)
            fe = np.zeros((L, FE), np.float32)
            fe[:n, 0:F] = features[cl]
            fe[:n, F] = 1.0
            shuf = (np.arange(L).reshape(nch, P).T).reshape(-1)  # row p*nch+c <- cand c*128+p
            gtabT[o : o + L] = gt[shuf]
            feat[o : o + L] = fe[shuf]
            outperm.append(lf)
        per_core.append({"qrow": qrow, "gtabT": gtabT, "feat": feat})
    outperm = np.concatenate(outperm)
    inv_perm = np.empty(B, np.int64)
    inv_perm[outperm] = np.arange(B)
    meta = {
        "sched": tuple(int(x) for x in sched),
        "offs": offs,
        "suml": suml,
        "cshift": cshift,
        "inv_perm": inv_perm,
    }
    return per_core, meta


# ------------------------------------------------------------- device build
def _build_nc(sched, suml, cshift, loop=1):
    nc = bacc.Bacc("TRN2")
    qrow_in = nc.declare_dram_parameter("qrow", [NSLOT, 3 * P], FP, isOutput=False)
    gtabT_in = nc.declare_dram_parameter("gtabT", [suml, 4], FP, isOutput=False)
    feat_in = nc.declare_dram_parameter("feat", [suml, FE], FP, isOutput=False)
    ident_in = nc.declare_dram_parameter("ident", [P, P], FP, isOutput=False)
    out = nc.declare_dram_parameter("out", [QPC, F], FP, isOutput=True)

    offs = np.concatenate([[0], np.cumsum(sched)]).astype(np.int64)

    with TileContext(nc) as tc:
        with tc.tile_pool(name="const", bufs=1) as cpool:
            ident = cpool.tile([P, P], FP)
            nc.sync.dma_start(ident[:], ident_in[:])
            cbias = cpool.tile([P, 1], FP)
            nc.vector.memset(cbias[:], cshift)

            with (
                tc.tile_pool(name="io", bufs=3) as io,
                tc.tile_pool(name="work", bufs=2) as wk,
                tc.tile_pool(name="chk", bufs=4) as ck,
                tc.tile_pool(name="ps_y", bufs=3, space="PSUM") as psy,
                tc.tile_pool(name="ps_b", bufs=2, space="PSUM") as psb,
                tc.tile_pool(name="ps_s", bufs=2, space="PSUM") as pss,
            ):
                for it in range(NSLOT * loop):
                    t = it % NSLOT
                    L = int(sched[t])
                    o = int(offs[t])
                    nch = L // P  # 128-wide sub-chunks
                    ng = (L + 511) // 512  # 512-wide groups

                    # ---- loads ----
                    csrc = io.tile([1, 3 * P], FP, tag="csrc", name=f"cs{it}")
                    nc.sync.dma_start(csrc[:], qrow_in[t : t + 1, :])
                    crep = wk.tile([P, 3 * P], FP, tag="crep", name=f"cr{it}")
                    nc.gpsimd.partition_broadcast(crep[:], csrc[:])
                    pcol = io.tile([P, nch, 4], FP, tag="pcol", name=f"pc{it}")
                    nc.sync.dma_start(
                        pcol[:],
                        gtabT_in[o : o + L, :].rearrange("(p c) f -> p c f", p=P),
                    )
                    ft = io.tile([P, nch, FE], FP, tag="ft", name=f"ft{it}")
                    nc.sync.dma_start(
                        ft[:],
                        feat_in[o : o + L, :].rearrange("(p c) f -> p c f", p=P),
                    )

                    yT = wk.tile([P, nch, P], FP, tag="yT", name=f"yT{it}")
                    e8all = wk.tile([P, 8 * ng], FP, tag="e8all", name=f"e8a{it}")

                    # ---- phase A: scores (exact direct differences) ----
                    for g in range(ng):
                        gw = min(4, nch - 4 * g)
                        psY = psy.tile([P, 512], FP, tag="psY", name=f"psY{it}_{g}")
                        for kk in range(gw):
                            ci = 4 * g + kk
                            sqa = ck.tile([P, P], FP, tag="sqa", name=f"sqa{it}_{ci}")
                            sqb = ck.tile([P, P], FP, tag="sqb", name=f"sqb{it}_{ci}")
                            nc.scalar.activation(
                                sqa[:], crep[:, 0:P], AF.Square,
                                bias=pcol[:, ci, 1:2],
                            )
                            nc.scalar.activation(
                                sqb[:], crep[:, P : 2 * P], AF.Square,
                                bias=pcol[:, ci, 2:3],
                            )
                            nc.vector.tensor_add(sqa[:], sqa[:], sqb[:])
                            nc.scalar.activation(
                                sqb[:], crep[:, 2 * P : 3 * P], AF.Square,
                                bias=pcol[:, ci, 3:4],
                            )
                            nc.vector.tensor_add(sqa[:], sqa[:], sqb[:])
                            nc.vector.tensor_scalar_mul(
                                yT[:, ci, :], sqa[:], pcol[:, ci, 0:1]
                            )
                            nc.tensor.transpose(
                                psY[:, kk * P : (kk + 1) * P], yT[:, ci, :], ident[:]
                            )
                        nc.vector.max(e8all[:, 8 * g : 8 * g + 8], psY[:, 0 : gw * P])

                    # ---- combine top-8; broadcast threshold ----
                    e8 = ck.tile([P, 8], FP, tag="e8", name=f"e8{it}")
                    nc.vector.max(e8[:], e8all[:])
                    psS = pss.tile([1, P], FP, tag="psS", name=f"psS{it}")
                    nc.tensor.transpose(psS[:], e8[:, 7:8], ident[:])
                    srow = ck.tile([1, P], FP, tag="srow", name=f"sr{it}")
                    nc.scalar.copy(srow[:], psS[:])
                    srep = wk.tile([P, P], FP, tag="srep", name=f"srp{it}")
                    nc.gpsimd.partition_broadcast(srep[:], srow[:])

                    # ---- phase B: masked exp weights + feature blend ----
                    psB = psb.tile([P, F + 1], FP, tag="psB", name=f"psB{it}")
                    for ci in range(nch):
                        msk = ck.tile([P, P], FP, tag="msk", name=f"m{it}_{ci}")
                        nc.vector.tensor_tensor(
                            out=msk[:], in0=yT[:, ci, :], in1=srep[:],
                            op=mybir.AluOpType.is_ge,
                        )
                        et = ck.tile([P, P], FP, tag="et", name=f"e{it}_{ci}")
                        nc.scalar.activation(
                            et[:], yT[:, ci, :], AF.Exp, bias=cbias[:]
                        )
                        nc.gpsimd.tensor_mul(et[:], et[:], msk[:])
                        nc.tensor.matmul(
                            psB[:],
                            et[:],
                            ft[:, ci, 0 : F + 1],
                            start=(ci == 0),
                            stop=(ci == nch - 1),
                        )

                    # ---- normalize + store ----
                    ob = ck.tile([P, F + 1], FP, tag="ob", name=f"ob{it}")
                    nc.scalar.copy(ob[:], psB[:])
                    rs = ck.tile([P, 1], FP, tag="rs", name=f"rs{it}")
                    nc.vector.reciprocal(rs[:], ob[:, F : F + 1])
                    ot = ck.tile([P, F], FP, tag="ot", name=f"ot{it}")
                    nc.vector.tensor_scalar_mul(ot[:], ob[:, 0:F], rs[:])
                    nc.sync.dma_start(out[ts(t, P), :], ot[:])

    nc.compile()
    return nc


# ------------------------------------------------------------------ runtime
_CACHE = {}


def _get_nc(sched, suml, cshift, loop):
    key = (sched, suml, round(cshift, 6), loop)
    if key not in _CACHE:
        _CACHE[key] = _build_nc(sched, suml, cshift, loop=loop)
    return _CACHE[key]


def make_in_maps(per_core):
    ident = np.eye(P, dtype=np.float32)
    return [
        {
            "qrow": pc["qrow"],
            "gtabT": pc["gtabT"],
            "feat": pc["feat"],
            "ident": ident,
        }
        for pc in per_core
    ]


LAST_RESULT = None


def kernel(coords, positions, weights, features):
    global LAST_RESULT
    import os

    per_core, meta = prep(coords, positions, weights, features)
    nc = _get_nc(meta["sched"], meta["suml"], meta["cshift"], LOOP)
    in_maps = make_in_maps(per_core)
    trace = bool(int(os.environ.get("KNN_TRACE", "0")))
    res = run_bass_kernel_spmd(nc, in_maps, core_ids=list(range(NCORES)), trace=trace)
    LAST_RESULT = res
    full = np.concatenate([res.results[i]["out"] for i in range(NCORES)], axis=0)
    return full[meta["inv_perm"]]


# revision 23
# speedup vs baseline: 1.2856x; 1.0830x over previous
"""Weighted-KNN (retrieval_knn) Trainium2 kernel — candidate-pruned, gather-free.

Math (per query c, over N anchors):
    sq[n]   = ||c - p_n||^2 / (w_n^2 + eps)
    top-8 smallest sq -> softmax(-sq_k / TEMP) -> weighted sum of features.

Strategy:
  HOST (numpy, in kernel()):
    * kd-split the 65536 queries into 512 spatial tiles of 128.
    * Per tile, an exact interval bound (f64) selects the candidate anchors
      that can possibly be in ANY tile query's top-8:
          T8 = 8th-smallest over anchors of max_{x in bbox} eff(x,n),
          keep n with min_{x in bbox} eff(x,n) <= T8.
      Mean ~320 candidates instead of 16384 (~39x less score work).
    * Tiles are LPT-balanced across the 8 cores (64 slots each), sorted by
      candidate count, and padded to one shared static schedule so a single
      NEFF serves all cores. Host ships per-core tables:
          qrow  [64, 384]   tile query coords (centered), rows c0|c1|c2
          gtabT [SUML, 4]   per-candidate [g0, -p'0, -p'1, -p'2],
                            g0 = -INV_TEMP/(w^2+eps)
          feat  [SUML, 72]  [features(64), 1.0, pad(7)]
  DEVICE (per tile, all engines pipelined, no dma_gather anywhere):
    * crep = partition_broadcast of the tile's query rows (Pool).
    * Scores via EXACT direct differences (same precision class as the
      reference): sq_d = Square(crep_d + (-p'_d)) on ScalarE with
      per-partition bias; y^T[j,q] = (sq0+sq1+sq2) * g0_j  (DVE).
    * PE-transpose y^T -> y[q,j] in PSUM; top-8 per query via DVE max8
      (per-512 pre-max8 + combine; exact fp32, pigeonhole-safe).
    * Mask  = (y^T >= s_bcast) (Pool), W = exp(y^T + C) * Mask
      (ScalarE exp + Pool mult; C is a global shift, cancels in softmax).
    * Feature blend as accumulating matmuls: out[q,:65] = sum_j W^T f_j,
      column 64 of feat is 1.0 so out[:,64] = Z (self-consistent softmax).
    * out = psB[:, :64] * (1/Z); DMA to DRAM; host un-permutes rows.
"""

import sys

if "/opt/trn_rl_repo" not in sys.path:
    sys.path.insert(0, "/opt/trn_rl_repo")

import numpy as np

import concourse.bacc as bacc
import concourse.bass as bass
import concourse.mybir as mybir
from concourse.bass import ts
from concourse.bass_utils import run_bass_kernel_spmd
from concourse.tile import TileContext

B, N, D, F = 65536, 16384, 3, 64
K = 8
BANDWIDTH = 0.05
TEMP = 2.0 * BANDWIDTH * BANDWIDTH  # 0.005
INV_TEMP = 1.0 / TEMP  # 200.0
EPS = 1e-8
NCORES = 8
QPC = B // NCORES  # 8192 queries per core
P = 128
NSLOT = QPC // P  # 64 tiles per core
FE = F + 8  # feat row: 64 features, ones col, 7 pad
LOOP = 1  # in-NEFF repetitions of the whole tile loop (benchmarking)

FP = mybir.dt.float32
AF = mybir.ActivationFunctionType


# ---------------------------------------------------------------- host prep
def _kd_leaves(coords):
    def split(idx, depth):
        if len(idx) == P:
            return [idx]
        ax = depth % 3
        k = len(idx) // 2
        part = np.argpartition(coords[idx, ax], k)
        return split(idx[part[:k]], depth + 1) + split(idx[part[k:]], depth + 1)

    return split(np.arange(coords.shape[0]), 0)


def prep(coords, positions, weights, features):
    """Host-side index construction. Returns (in_maps_arrays, meta)."""
    coords = np.ascontiguousarray(coords, dtype=np.float32)
    p64 = np.ascontiguousarray(positions, dtype=np.float64)
    w64 = np.ascontiguousarray(weights, dtype=np.float64)
    features = np.ascontiguousarray(features, dtype=np.float32)
    inv64 = 1.0 / (w64 * w64 + EPS)

    leaves = _kd_leaves(coords)
    ntiles = len(leaves)
    cands, centers, counts, t8s = [], [], [], []
    for lf in leaves:
        c = coords[lf].astype(np.float64)
        lo, hi = c.min(0), c.max(0)
        dmin2 = (np.clip(np.maximum(lo - p64, p64 - hi), 0, None) ** 2).sum(1)
        dmax2 = (np.maximum((p64 - lo) ** 2, (p64 - hi) ** 2)).sum(1)
        emin, emax = dmin2 * inv64, dmax2 * inv64
        t8 = np.partition(emax, K - 1)[K - 1] * (1 + 1e-4) + 1e-9
        cl = np.where(emin <= t8)[0]
        assert len(cl) >= K
        cands.append(cl)
        centers.append((lo + hi) / 2)
        counts.append(len(cl))
        t8s.append(t8)
    counts = np.array(counts)
    t8s = np.array(t8s)
    assert INV_TEMP * t8s.max() <= 160.0, t8s.max()
    cshift = float(np.clip(INV_TEMP * t8s.max() - 40.0, 0.0, 80.0))

    # LPT-balance tiles onto cores (64 slots each), sort desc by count
    order = np.argsort(-counts, kind="stable")
    loads = np.zeros(NCORES)
    slots = [[] for _ in range(NCORES)]
    for t in order:
        free = [c for c in range(NCORES) if len(slots[c]) < NSLOT]
        c = min(free, key=lambda c: loads[c])
        slots[c].append(t)
        loads[c] += counts[t]
    for c in range(NCORES):
        slots[c].sort(key=lambda t: -counts[t])
    ls = np.array(
        [[counts[slots[c][j]] for j in range(NSLOT)] for c in range(NCORES)]
    )
    sched = np.maximum(P, ((ls.max(0) + P - 1) // P) * P).astype(np.int64)
    offs = np.concatenate([[0], np.cumsum(sched)])
    suml = int(offs[-1])

    per_core = []
    outperm = []
    for c in range(NCORES):
        qrow = np.zeros((NSLOT, 3 * P), np.float32)
        gtabT = np.zeros((suml, 4), np.float32)
        gtabT[:, 0] = -1.0
        gtabT[:, 1:4] = -100.0  # pad: far away, y ~ -3e4
        feat = np.zeros((suml, FE), np.float32)
        for j in range(NSLOT):
            t = slots[c][j]
            lf, cl, ctr = leaves[t], cands[t], centers[t]
            qc = (coords[lf].astype(np.float64) - ctr).astype(np.float32)
            qrow[j, :] = qc.T.reshape(-1)
            o, n = offs[j], len(cl)
            L = int(sched[j])
            nch = L // P
            # device reads tile slices as [128, nch*row] with partition p
            # holding DRAM rows [o + p*nch, o + (p+1)*nch) contiguously; host
            # permutes so candidate j = c*128 + p lands at row o + p*nch + c.
            gt = np.zeros((L, 4), np.float32)
            gt[:, 0] = -1.0
            gt[:, 1:4] = -100.0
            gt[:n, 0] = (-INV_TEMP * inv64[cl]).astype(np.float32)
            gt[:n, 1:4] = -(p64[cl] - ctr).astype(np.float32)
            fe = np.zeros((L, FE), np.float32)
            fe[:n, 0:F] = features[cl]
            fe[:n, F] = 1.0
            shuf = (np.arange(L).reshape(nch, P).T).reshape(-1)  # row p*nch+c <- cand c*128+p
            gtabT[o : o + L] = gt[shuf]
            feat[o : o + L] = fe[shuf]
            outperm.append(lf)
        per_core.append({"qrow": qrow, "gtabT": gtabT, "feat": feat})
    outperm = np.concatenate(outperm)
    inv_perm = np.empty(B, np.int64)
    inv_perm[outperm] = np.arange(B)
    meta = {
        "sched": tuple(int(x) for x in sched),
        "offs": offs,
        "suml": suml,
        "cshift": cshift,
        "inv_perm": inv_perm,
    }
    return per_core, meta


# ------------------------------------------------------------- device build
def _build_nc(sched, suml, cshift, loop=1):
    nc = bacc.Bacc("TRN2")
    qrow_in = nc.declare_dram_parameter("qrow", [NSLOT, 3 * P], FP, isOutput=False)
    gtabT_in = nc.declare_dram_parameter("gtabT", [suml, 4], FP, isOutput=False)
    feat_in = nc.declare_dram_parameter("feat", [suml, FE], FP, isOutput=False)
    ident_in = nc.declare_dram_parameter("ident", [P, P], FP, isOutput=False)
    out = nc.declare_dram_parameter("out", [QPC, F], FP, isOutput=True)

    offs = np.concatenate([[0], np.cumsum(sched)]).astype(np.int64)

    with TileContext(nc) as tc:
        with tc.tile_pool(name="const", bufs=1) as cpool:
            ident = cpool.tile([P, P], FP)
            nc.sync.dma_start(ident[:], ident_in[:])
            cbias = cpool.tile([P, 1], FP)
            nc.vector.memset(cbias[:], cshift)

            with (
                tc.tile_pool(name="io", bufs=3) as io,
                tc.tile_pool(name="work", bufs=3) as wk,
                tc.tile_pool(name="chk", bufs=4) as ck,
                tc.tile_pool(name="ps_y", bufs=4, space="PSUM") as psy,
                tc.tile_pool(name="ps_b", bufs=2, space="PSUM") as psb,
                tc.tile_pool(name="ps_s", bufs=2, space="PSUM") as pss,
            ):
                for it in range(NSLOT * loop):
                    t = it % NSLOT
                    L = int(sched[t])
                    o = int(offs[t])
                    nch = L // P  # 128-wide sub-chunks
                    ng = (L + 511) // 512  # 512-wide groups

                    # ---- loads ----
                    csrc = io.tile([1, 3 * P], FP, tag="csrc", name=f"cs{it}")
                    nc.sync.dma_start(csrc[:], qrow_in[t : t + 1, :])
                    crep = wk.tile([P, 3 * P], FP, tag="crep", name=f"cr{it}")
                    nc.gpsimd.partition_broadcast(crep[:], csrc[:])
                    pcol = io.tile([P, nch, 4], FP, tag="pcol", name=f"pc{it}")
                    nc.sync.dma_start(
                        pcol[:],
                        gtabT_in[o : o + L, :].rearrange("(p c) f -> p c f", p=P),
                    )
                    ft = io.tile([P, nch, FE], FP, tag="ft", name=f"ft{it}")
                    nc.sync.dma_start(
                        ft[:],
                        feat_in[o : o + L, :].rearrange("(p c) f -> p c f", p=P),
                    )

                    yT = wk.tile([P, nch, P], FP, tag="yT", name=f"yT{it}")
                    e8all = wk.tile([P, 8 * ng], FP, tag="e8all", name=f"e8a{it}")

                    # ---- phase A: scores (exact direct differences) ----
                    for g in range(ng):
                        gw = min(4, nch - 4 * g)
                        psY = psy.tile([P, 512], FP, tag="psY", name=f"psY{it}_{g}")
                        for kk in range(gw):
                            ci = 4 * g + kk
                            sqa = ck.tile([P, P], FP, tag="sqa", name=f"sqa{it}_{ci}")
                            sqb = ck.tile([P, P], FP, tag="sqb", name=f"sqb{it}_{ci}")
                            nc.scalar.activation(
                                sqa[:], crep[:, 0:P], AF.Square,
                                bias=pcol[:, ci, 1:2],
                            )
                            nc.scalar.activation(
                                sqb[:], crep[:, P : 2 * P], AF.Square,
                                bias=pcol[:, ci, 2:3],
                            )
                            nc.vector.tensor_add(sqa[:], sqa[:], sqb[:])
                            nc.scalar.activation(
                                sqb[:], crep[:, 2 * P : 3 * P], AF.Square,
                                bias=pcol[:, ci, 3:4],
                            )
                            nc.vector.tensor_add(sqa[:], sqa[:], sqb[:])
                            nc.vector.tensor_scalar_mul(
                                yT[:, ci, :], sqa[:], pcol[:, ci, 0:1]
                            )
                            nc.tensor.transpose(
                                psY[:, kk * P : (kk + 1) * P], yT[:, ci, :], ident[:]
                            )
                        nc.vector.max(e8all[:, 8 * g : 8 * g + 8], psY[:, 0 : gw * P])

                    # ---- combine top-8; broadcast threshold ----
                    e8 = ck.tile([P, 8], FP, tag="e8", name=f"e8{it}")
                    nc.vector.max(e8[:], e8all[:])
                    psS = pss.tile([1, P], FP, tag="psS", name=f"psS{it}")
                    nc.tensor.transpose(psS[:], e8[:, 7:8], ident[:])
                    srow = ck.tile([1, P], FP, tag="srow", name=f"sr{it}")
                    nc.scalar.copy(srow[:], psS[:])
                    srep = wk.tile([P, P], FP, tag="srep", name=f"srp{it}")
                    nc.gpsimd.partition_broadcast(srep[:], srow[:])

                    # ---- phase B: masked exp weights + feature blend ----
                    psB = psb.tile([P, F + 1], FP, tag="psB", name=f"psB{it}")
                    for ci in range(nch):
                        msk = ck.tile([P, P], FP, tag="msk", name=f"m{it}_{ci}")
                        nc.vector.tensor_tensor(
                            out=msk[:], in0=yT[:, ci, :], in1=srep[:],
                            op=mybir.AluOpType.is_ge,
                        )
                        et = ck.tile([P, P], FP, tag="et", name=f"e{it}_{ci}")
                        nc.scalar.activation(
                            et[:], yT[:, ci, :], AF.Exp, bias=cbias[:]
                        )
                        nc.gpsimd.tensor_mul(et[:], et[:], msk[:])
                        nc.tensor.matmul(
                            psB[:],
                            et[:],
                            ft[:, ci, 0 : F + 1],
                            start=(ci == 0),
                            stop=(ci == nch - 1),
                        )

                    # ---- normalize + store ----
                    ob = ck.tile([P, F + 1], FP, tag="ob", name=f"ob{it}")
                    nc.scalar.copy(ob[:], psB[:])
                    rs = ck.tile([P, 1], FP, tag="rs", name=f"rs{it}")
                    nc.vector.reciprocal(rs[:], ob[:, F : F + 1])
                    ot = ck.tile([P, F], FP, tag="ot", name=f"ot{it}")
                    nc.vector.tensor_scalar_mul(ot[:], ob[:, 0:F], rs[:])
                    nc.sync.dma_start(out[ts(t, P), :], ot[:])

    nc.compile()
    return nc


# ------------------------------------------------------------------ runtime
_CACHE = {}


def _get_nc(sched, suml, cshift, loop):
    key = (sched, suml, round(cshift, 6), loop)
    if key not in _CACHE:
        _CACHE[key] = _build_nc(sched, suml, cshift, loop=loop)
    return _CACHE[key]


def make_in_maps(per_core):
    ident = np.eye(P, dtype=np.float32)
    return [
        {
            "qrow": pc["qrow"],
            "gtabT": pc["gtabT"],
            "feat": pc["feat"],
            "ident": ident,
        }
        for pc in per_core
    ]


LAST_RESULT = None


def kernel(coords, positions, weights, features):
    global LAST_RESULT
    import os

    per_core, meta = prep(coords, positions, weights, features)
    nc = _get_nc(meta["sched"], meta["suml"], meta["cshift"], LOOP)
    in_maps = make_in_maps(per_core)
    trace = bool(int(os.environ.get("KNN_TRACE", "0")))
    res = run_bass_kernel_spmd(nc, in_maps, core_ids=list(range(NCORES)), trace=trace)
    LAST_RESULT = res
    full = np.concatenate([res.results[i]["out"] for i in range(NCORES)], axis=0)
    return full[meta["inv_perm"]]
